# revision 8
# baseline (speedup 1.0000x reference)
"""CrossAttention Trainium2 Bass kernel.

Problem (hardcoded): B=16, Lq=Lk=2048, Dq=768, Dk=1024, fp32 reference.
  q = query @ Wq + bq ; k = key @ Wk + bk ; v = key @ Wv + bv
  out = softmax(q k^T / sqrt(1024)) @ v

Sharding: data-parallel over batch, 2 batches per core on 8 cores. The
full [16, ...] host arrays are exactly the concatenation of the 8
per-core shards, so shard_map's P("core") sharding needs no host-side
concat/split at all.

A warm call is dominated by host<->device transfer bytes plus per-call
dispatch overhead, not device compute (~0.9 ms), so:
  - query/key/out travel as fp16 (half the bytes; rel err ~5e-4 on HW
    vs the 2e-2 gate), cast host-side with jitted multi-threaded
    jax-CPU converts (4-10x faster than np.astype).
  - Projection weights are embedded in the NEFF as Const tensors
    (nc.inline_tensor): zero per-call bytes, no 8x per-core duplication.
    The build is cached on a hash of the weight bytes, so changed
    weights trigger a correct (slow) rebuild.
  - The jit(shard_map(bass_exec)) executable is built once and cached;
    warm calls skip retracing/lowering.  This also drops the donated
    zero-output upload that bass2jax.run_bass_via_pjrt would pay every
    call (the bass_exec NEFF path never reads those buffers; every
    output element is written by the kernel).
    Set XATTN_RUNNER=spmd to route through bass_utils.run_bass_kernel_spmd
    instead (same NEFF, slower dispatch).

Device side (per core, 2 batches serial; ~0.87 ms predicted by
TimelineSim, ~91% PE occupancy):
  - All matmuls fp16: full 78.6 TF/s PE rate with fp32 PSUM accumulation.
  - qT, kT, v SBUF-resident per batch -- no DRAM spills (the fp32
    baseline spilled qT/keyT to DRAM and reloaded them).
  - Weights SBUF-resident across both batches (one DMA each).
  - ACT runs only exp (no activation-table switching); normalization
    scale and +bv run on DVE.
  - Row sums of exp(scores) via the ones-column matmul trick (sums land
    per-q-row-partition, aligned with the output PSUM tile).

Math simplifications (exact up to rounding):
  - bk shifts every score row by a per-query constant -> cancels in
    softmax, dropped entirely.
  - softmax weights sum to 1 -> bv passes through attention unchanged:
    added once to the final output.
  - scores/32 are bounded (|s|<~3) so exp() without max-subtraction is
    safe (fp32 PSUM, fp16 exp output, fp32 row sums).
"""

import hashlib
import os
import numpy as np

B, LQ, LK = 16, 2048, 2048
DQ, DK = 768, 1024
N_CORES = 8
BPC = B // N_CORES  # batches per core

DT = os.environ.get("XATTN_DT", "float16")
RUNNER = os.environ.get("XATTN_RUNNER", "jit")  # jit | spmd
# Ship query as fp8 e3m4 (scores-only path; ~25MB less wire, ~3e-3 rel err).
Q8 = os.environ.get("XATTN_Q8", "0") == "1"


def build_nc(weights, bpc=BPC, lq=LQ, lk=LK, dt_name=DT, c_t=512, reps=1,
             q8=Q8):
    import concourse.bass as bass
    import concourse.mybir as mybir
    from concourse import bacc
    import concourse.tile as tile
    from concourse.masks import make_identity

    fp32 = mybir.dt.float32
    mdt = getattr(mybir.dt, dt_name)
    KCQ = DQ // 128   # 6 contraction chunks for q projection
    KCK = DK // 128   # 8 contraction chunks for k/v projection + scores
    NLK = lk // 128   # Lk subtiles of 128
    NCQ = lq // c_t   # Lq tiles (attention phase)
    CS = c_t // 128   # Lq subtiles per attention tile

    nc = bacc.Bacc("TRN2")
    qdt = mybir.dt.float8e3 if q8 else mdt
    query = nc.dram_tensor("query", [bpc, lq, DQ], qdt, kind="ExternalInput")
    key = nc.dram_tensor("key", [bpc, lk, DK], mdt, kind="ExternalInput")
    out = nc.dram_tensor("out", [bpc, lq, DK], mdt, kind="ExternalOutput")
    Wq = nc.inline_tensor(weights["Wq"], name="Wq")
    bq = nc.inline_tensor(weights["bq"], name="bq")
    Wk = nc.inline_tensor(weights["Wk"], name="Wk")
    Wv = nc.inline_tensor(weights["Wv"], name="Wv")
    bv = nc.inline_tensor(weights["bv"], name="bv")

    def mm(ps, lhsT, rhs, start, stop):
        nc.tensor.matmul(ps, lhsT, rhs, start=start, stop=stop)

    with tile.TileContext(nc) as tc:
        with (
            tc.tile_pool(name="const", bufs=1) as constp,
            tc.tile_pool(name="weights", bufs=1) as wp,
            tc.tile_pool(name="kT", bufs=1) as kTp,
            tc.tile_pool(name="v", bufs=1) as vp,
            tc.tile_pool(name="qT", bufs=1) as qTp,
        ):
            ident_f32 = constp.tile([128, 128], fp32)
            make_identity(nc, ident_f32)
            ident = constp.tile([128, 128], mdt)
            nc.vector.tensor_copy(ident, ident_f32)
            ones_f32 = constp.tile([128, 4], fp32)
            nc.vector.memset(ones_f32, 1.0)
            ones_col = constp.tile([128, 4], mdt)
            nc.vector.tensor_copy(ones_col, ones_f32)
            bq_sb = constp.tile([128, KCK], fp32)
            nc.sync.dma_start(bq_sb, bq.rearrange("(c p) -> p c", p=128))
            bv_rep = constp.tile([128, DK], fp32)
            nc.sync.dma_start(bv_rep, bv[None, :].partition_broadcast(128))

            # Weights resident in SBUF for the whole kernel (one DMA each).
            wq_sb = wp.tile([128, KCQ, DK], mdt)
            nc.sync.dma_start(wq_sb, Wq.rearrange("(c p) n -> p c n", p=128))
            wk_sb = wp.tile([128, KCK, DK], mdt)
            nc.sync.dma_start(wk_sb, Wk.rearrange("(c p) n -> p c n", p=128))
            wv_sb = wp.tile([128, KCK, DK], mdt)
            nc.sync.dma_start(wv_sb, Wv.rearrange("(c p) n -> p c n", p=128))

            for b in [bb for _ in range(reps) for bb in range(bpc)]:
                kT_sb = kTp.tile([128, KCK, lk], mdt)   # kT[dk, lk]
                v_sb = vp.tile([128, NLK, DK], mdt)     # v[lk, dk]
                qT_sb = qTp.tile([128, KCK, lq], mdt)   # qT[dk, lq]

                # ---- Phase K: per 512-row key tile: transpose -> kT, v ----
                with (
                    tc.tile_pool(name="kproj", bufs=2) as kp,
                    tc.tile_pool(name="kps_t", bufs=2, space="PSUM") as kps_t,
                    tc.tile_pool(name="kps_k", bufs=2, space="PSUM") as kps_k,
                    tc.tile_pool(name="kps_v", bufs=2, space="PSUM") as kps_v,
                ):
                    for t in range(lk // 512):
                        kn = kp.tile([128, 4, DK], mdt, tag="knat")
                        nc.sync.dma_start(
                            kn,
                            key[b, t * 512:(t + 1) * 512, :].rearrange(
                                "(s p) d -> p s d", p=128
                            ),
                        )
                        kTt = kp.tile([128, KCK, 512], mdt, tag="kTt")
                        for s in range(4):
                            for kc in range(KCK):
                                ps = kps_t.tile([128, 128], mdt, tag="tp")
                                nc.tensor.transpose(
                                    ps, kn[:, s, kc * 128:(kc + 1) * 128], ident
                                )
                                nc.vector.tensor_copy(
                                    kTt[:, kc, s * 128:(s + 1) * 128], ps
                                )
                        for mc in range(KCK):
                            ps = kps_k.tile([128, 512], fp32, tag="kmm")
                            for kc in range(KCK):
                                mm(ps, wk_sb[:, kc, mc * 128:(mc + 1) * 128],
                                   kTt[:, kc, :], kc == 0, kc == KCK - 1)
                            nc.vector.tensor_copy(
                                kT_sb[:, mc, t * 512:(t + 1) * 512], ps
                            )
                        for s in range(4):
                            for dk in range(2):
                                ps = kps_v.tile([128, 512], fp32, tag="vmm")
                                for kc in range(KCK):
                                    mm(ps, kTt[:, kc, s * 128:(s + 1) * 128],
                                       wv_sb[:, kc, dk * 512:(dk + 1) * 512],
                                       kc == 0, kc == KCK - 1)
                                nc.vector.tensor_copy(
                                    v_sb[:, t * 4 + s, dk * 512:(dk + 1) * 512],
                                    ps,
                                )

                # ---- Phase Q: qT = Wq^T queryT + bq, SBUF resident ----
                with (
                    tc.tile_pool(name="qproj", bufs=2) as qp,
                    tc.tile_pool(name="qps_t", bufs=2, space="PSUM") as qps_t,
                    tc.tile_pool(name="qps_m", bufs=2, space="PSUM") as qps_m,
                ):
                    for t in range(lq // 512):
                        qn_in = qp.tile([128, 4, DQ], qdt, tag="qn_in")
                        nc.sync.dma_start(
                            qn_in,
                            query[b, t * 512:(t + 1) * 512, :].rearrange(
                                "(s p) d -> p s d", p=128
                            ),
                        )
                        if q8:
                            qn = qp.tile([128, 4, DQ], mdt, tag="qnat")
                            nc.vector.tensor_copy(qn, qn_in)
                        else:
                            qn = qn_in
                        qTt = qp.tile([128, KCQ, 512], mdt, tag="qTt")
                        for s in range(4):
                            for kc in range(KCQ):
                                ps = qps_t.tile([128, 128], mdt, tag="tp")
                                nc.tensor.transpose(
                                    ps, qn[:, s, kc * 128:(kc + 1) * 128], ident
                                )
                                nc.vector.tensor_copy(
                                    qTt[:, kc, s * 128:(s + 1) * 128], ps
                                )
                        for mc in range(KCK):
                            ps = qps_m.tile([128, 512], fp32, tag="qmm")
                            for kc in range(KCQ):
                                mm(ps, wq_sb[:, kc, mc * 128:(mc + 1) * 128],
                                   qTt[:, kc, :], kc == 0, kc == KCQ - 1)
                            nc.vector.tensor_scalar_add(
                                qT_sb[:, mc, t * 512:(t + 1) * 512], ps,
                                bq_sb[:, mc:mc + 1],
                            )

                # ---- Phase C: attention ----
                with (
                    tc.tile_pool(name="attn", bufs=2) as cp,
                    tc.tile_pool(name="expp", bufs=NLK + 2) as ep,
                    tc.tile_pool(name="cps_s", bufs=2, space="PSUM") as cps_s,
                    tc.tile_pool(name="cps_o", bufs=2, space="PSUM") as cps_o,
                    tc.tile_pool(name="cps_n", bufs=2, space="PSUM") as cps_n,
                ):
                    for t in range(NCQ):
                        exps = []
                        for lkb in range(NLK):
                            ps_s = cps_s.tile([128, c_t], fp32, tag="sc")
                            for kc in range(KCK):
                                mm(ps_s, kT_sb[:, kc, lkb * 128:(lkb + 1) * 128],
                                   qT_sb[:, kc, t * c_t:(t + 1) * c_t],
                                   kc == 0, kc == KCK - 1)
                            ex = ep.tile([128, c_t], mdt, tag="exp")
                            nc.scalar.activation(
                                ex, ps_s, mybir.ActivationFunctionType.Exp,
                                scale=1.0 / 32.0,
                            )
                            exps.append(ex)
                        for s in range(CS):
                            ps_o = cps_o.tile([128, DK], fp32, tag="pv")
                            ps_n = cps_n.tile([128, 4], fp32, tag="sum")
                            for lkb in range(NLK):
                                lhs = exps[lkb][:, s * 128:(s + 1) * 128]
                                for dk in range(2):
                                    mm(ps_o[:, dk * 512:(dk + 1) * 512], lhs,
                                       v_sb[:, lkb, dk * 512:(dk + 1) * 512],
                                       lkb == 0, lkb == NLK - 1)
                                mm(ps_n, lhs, ones_col, lkb == 0, lkb == NLK - 1)
                            rec = cp.tile([128, 1], fp32, tag="rec")
                            nc.vector.reciprocal(rec, ps_n[:, 0:1])
                            o_f32 = cp.tile([128, DK], fp32, tag="osb")
                            nc.vector.tensor_scalar_mul(o_f32, ps_o, rec[:, 0:1])
                            o16 = cp.tile([128, DK], mdt, tag="o16")
                            nc.vector.tensor_add(o16, o_f32, bv_rep)
                            nc.sync.dma_start(
                                out[b, t * c_t + s * 128:t * c_t + (s + 1) * 128, :],
                                o16,
                            )
    return nc


_NC_CACHE = {}
_RUNNER_CACHE = {}
_CAST_CACHE = {}


def _np_dt():
    import ml_dtypes

    return {"float16": np.float16, "bfloat16": ml_dtypes.bfloat16}[DT]


def _cpu_cast(x, np_dtype):
    """Multi-threaded dtype cast on the jax CPU backend (4-10x np.astype)."""
    import jax

    x = np.asarray(x)
    if x.dtype == np_dtype:
        return np.ascontiguousarray(x)
    key = np.dtype(np_dtype).name
    if key not in _CAST_CACHE:
        import jax.numpy as jnp

        jdt = jnp.dtype(np_dtype)
        _CAST_CACHE[key] = jax.jit(lambda a: a.astype(jdt))
    cpu = jax.devices("cpu")[0]
    with jax.default_device(cpu):
        return np.asarray(_CAST_CACHE[key](x))


def _weights_key(weights):
    h = hashlib.blake2b(digest_size=16)
    for n in ("Wq", "bq", "Wk", "Wv", "bv"):
        h.update(weights[n].tobytes())
    return (h.hexdigest(), DT, Q8)


def _get_nc(inputs):
    dt = _np_dt()
    weights = {
        "Wq": _cpu_cast(inputs["Wq"], dt),
        "Wk": _cpu_cast(inputs["Wk"], dt),
        "Wv": _cpu_cast(inputs["Wv"], dt),
        "bq": _cpu_cast(inputs["bq"], np.float32),
        "bv": _cpu_cast(inputs["bv"], np.float32),
    }
    key = _weights_key(weights)
    if key not in _NC_CACHE:
        nc = build_nc(weights)
        nc.finalize()
        _NC_CACHE[key] = nc
    return key, _NC_CACHE[key]


def _make_runner(nc):
    """jit(shard_map(bass_exec)) built once per nc.

    Mirrors bass2jax.run_bass_via_pjrt's multi-core branch minus the
    per-call retrace and the donated zero-output operands (the bass_exec
    NEFF never reads those; this kernel writes every output element)."""
    import jax
    from jax.sharding import Mesh, PartitionSpec
    from jax.experimental.shard_map import shard_map
    from concourse import bass2jax, mybir

    bass2jax.install_neuronx_cc_hook()
    partition_name = (
        nc.partition_id_tensor.name if nc.partition_id_tensor else None
    )
    in_names, out_names, out_avals = [], [], []
    for alloc in nc.m.functions[0].allocations:
        if not isinstance(alloc, mybir.MemoryLocationSet):
            continue
        name = alloc.memorylocations[0].name
        if alloc.kind == "ExternalInput":
            if name != partition_name:
                in_names.append(name)
        elif alloc.kind == "ExternalOutput":
            out_names.append(name)
            out_avals.append(
                jax.core.ShapedArray(
                    tuple(alloc.tensor_shape), mybir.dt.np(alloc.dtype)
                )
            )
    n_params = len(in_names)
    in_names_all = list(in_names)
    if partition_name is not None:
        in_names_all.append(partition_name)

    def _body(*args):
        operands = list(args)
        if partition_name is not None:
            operands.append(bass2jax.partition_id_tensor())
        outs = bass2jax._bass_exec_p.bind(
            *operands,
            out_avals=tuple(out_avals),
            in_names=tuple(in_names_all),
            out_names=tuple(out_names),
            lowering_input_output_aliases=(),
            sim_require_finite=True,
            sim_require_nnan=True,
            nc=nc,
        )
        return tuple(outs)

    devices = jax.devices()[:N_CORES]
    assert len(devices) == N_CORES, (
        f"need {N_CORES} cores, visible: {len(devices)}"
    )
    mesh = Mesh(np.asarray(devices), ("core",))
    return jax.jit(
        shard_map(
            _body, mesh=mesh,
            in_specs=(PartitionSpec("core"),) * n_params,
            out_specs=(PartitionSpec("core"),) * len(out_names),
            check_rep=False,
        )
    )


def kernel(**inputs):
    import ml_dtypes

    dt = _np_dt()
    qdt = ml_dtypes.float8_e3m4 if Q8 else dt
    query = _cpu_cast(inputs["query"], qdt)
    key = _cpu_cast(inputs["key"], dt)
    ckey, nc = _get_nc(inputs)

    if RUNNER == "spmd":
        from concourse.bass_utils import run_bass_kernel_spmd

        in_maps = [
            {"query": query[c * BPC:(c + 1) * BPC],
             "key": key[c * BPC:(c + 1) * BPC]}
            for c in range(N_CORES)
        ]
        res = run_bass_kernel_spmd(nc, in_maps, core_ids=list(range(N_CORES)))
        return np.concatenate(
            [_cpu_cast(r["out"], np.float32) for r in res.results], axis=0
        )

    if ckey not in _RUNNER_CACHE:
        _RUNNER_CACHE[ckey] = _make_runner(nc)
    (out16,) = _RUNNER_CACHE[ckey](query, key)
    return _cpu_cast(np.asarray(out16), np.float32)
